# revision 1
# baseline (speedup 1.0000x reference)
"""Trainium2 Bass kernel for CrossMerge3D.

Input ys: [B=2, S=12, C=96, 32, 32, 32] f32. For each (b, c):
  out = (m0 + perm_j(m1) + perm_k(m2)) / 12
where, with the 12 scans split into 3 groups of 4, each group combines as
  m_g = s0 + s1 + flip(s2 + s3)   (flip over the flattened 32^3 volume)
and group 1's volume is stored as (j,k,i), group 2's as (k,i,j); perm_j /
perm_k bring them back to (i,j,k).

Sharding: 8 cores = batch (2) x channel quarters (4) -> 24 channels/core.
No cross-core communication.

Per-core layout: 4 channels x 32 leading-spatial -> 128 SBUF partitions,
1024-wide free dim (the remaining two spatial axes). Scan pairs are summed
during the load via DMA-accumulate (SDMA CCE add); the in-pair flip is a
partition-block-reversed load plus a free-dim-reversed operand AP on the
combining add. perm_j / perm_k are DVE 32x32 block transposes plus
free-dim (a,b)->(b,a) permuted access patterns.
"""

import os
import numpy as np

_B, _S, _C, _D = 2, 12, 96, 32
_NCORE = 8
_CL = _C // 4          # 24 channels per core
_G = _CL // 4          # 6 macro tiles of 4 channels (128 partitions)
_FREE = _D * _D        # 1024

_nc = None


def _build_program():
    from concourse import bacc, tile, mybir

    f32 = mybir.dt.float32
    nc = bacc.Bacc(
        "TRN2", target_bir_lowering=False, debug=False, num_devices=_NCORE
    )
    ys = nc.dram_tensor("ys", [_S, _CL, _D, _D, _D], f32, kind="ExternalInput")
    out = nc.dram_tensor("out", [_CL, _D, _D, _D], f32, kind="ExternalOutput")
    ysa = ys.ap()
    outa = out.ap()

    with tile.TileContext(nc) as tc:
        with (
            tc.tile_pool(name="io", bufs=2) as iop,
            tc.tile_pool(name="tmp", bufs=2) as tmp,
        ):
            for g in range(_G):
                cs = slice(4 * g, 4 * (g + 1))

                def src(s, rev):
                    a = ysa[s, cs]  # [4, 32, 32, 32]
                    if rev:
                        # reverse the leading spatial axis; (c, i) can no
                        # longer merge, so keep a 3D AP
                        return a[:, ::-1].rearrange("c i j k -> c i (j k)")
                    return a.rearrange("c i j k -> (c i) (j k)")

                def pair(s, rev, tag):
                    # t = scan s + scan s+1, pair-summed by DMA-accumulate
                    t = iop.tile([128, _FREE], f32, tag=tag, name=tag)
                    nc.sync.dma_start(out=t[:], in_=src(s, rev))
                    nc.gpsimd.dma_start(
                        out=t[:],
                        in_=src(s + 1, rev),
                        accum_op=mybir.AluOpType.add,
                    )
                    return t

                af = pair(0, False, "af")
                ar = pair(2, True, "ar")
                bf = pair(4, False, "bf")
                br = pair(6, True, "br")
                cf = pair(8, False, "cf")
                cr = pair(10, True, "cr")

                def p3(ap):
                    return ap.rearrange("p (a b) -> p a b", a=_D)

                def pswap(ap):
                    return p3(ap).transpose([0, 2, 1])

                # group 0 (volume already (i,j,k)): acc = fwd + flip(rev)
                acc = tmp.tile([128, _FREE], f32, tag="acc", name="acc")
                nc.vector.tensor_add(acc[:], af[:], ar[:][:, ::-1])

                # group 1 ((j,k,i) order): combine, 32x32 block transpose,
                # then add with (k,j)->(j,k) free permute
                xb = tmp.tile([128, _FREE], f32, tag="xb", name="xb")
                nc.vector.tensor_add(xb[:], bf[:], br[:][:, ::-1])
                tb = tmp.tile([128, _FREE], f32, tag="tb", name="tb")
                nc.vector.transpose(tb[:], xb[:])
                nc.vector.tensor_add(p3(acc[:]), p3(acc[:]), pswap(tb[:]))

                # group 2 ((k,i,j) order): combine, (i,j)->(j,i) free
                # permute (ScalarE), then 32x32 block transpose
                xc = tmp.tile([128, _FREE], f32, tag="xc", name="xc")
                nc.vector.tensor_add(xc[:], cf[:], cr[:][:, ::-1])
                cp = tmp.tile([128, _FREE], f32, tag="cp", name="cp")
                nc.scalar.copy(p3(cp[:]), pswap(xc[:]))
                tcb = tmp.tile([128, _FREE], f32, tag="tcb", name="tcb")
                nc.vector.transpose(tcb[:], cp[:])
                nc.vector.tensor_add(acc[:], acc[:], tcb[:])

                o = tmp.tile([128, _FREE], f32, tag="o", name="o")
                nc.scalar.mul(o[:], acc[:], 1.0 / 12.0)
                nc.sync.dma_start(
                    out=outa[cs].rearrange("c i j k -> (c i) (j k)"), in_=o[:]
                )

    nc.compile()
    return nc


def kernel(ys):
    global _nc
    ys = np.ascontiguousarray(ys, dtype=np.float32)
    assert ys.shape == (_B, _S, _C, _D, _D, _D), ys.shape

    if _nc is None:
        _nc = _build_program()

    from concourse.bass_utils import run_bass_kernel_spmd

    in_maps = []
    for r in range(_NCORE):
        b, q = divmod(r, 4)
        shard = np.ascontiguousarray(ys[b, :, q * _CL:(q + 1) * _CL])
        in_maps.append({"ys": shard})

    res = run_bass_kernel_spmd(_nc, in_maps, list(range(_NCORE)))

    out = np.empty((_B, _C, _D, _D, _D), np.float32)
    for r in range(_NCORE):
        b, q = divmod(r, 4)
        out[b, q * _CL:(q + 1) * _CL] = res.results[r]["out"]

    if res.exec_time_ns is not None:
        print(f"HW exec time: {res.exec_time_ns} ns")
    return out


# revision 3
# speedup vs baseline: 1.0339x; 1.0339x over previous
"""Trainium2 Bass kernel for CrossMerge3D.

Input ys: [B=2, S=12, C=96, 32, 32, 32] f32. For each (b, c):
  out = (m0 + perm_j(m1) + perm_k(m2)) / 12
where, with the 12 scans split into 3 groups of 4, each group combines as
  m_g = s0 + s1 + flip(s2 + s3)   (flip over the flattened 32^3 volume)
and group 1's volume is stored as (j,k,i), group 2's as (k,i,j); perm_j /
perm_k bring them back to (i,j,k).

Sharding: 8 cores = batch (2) x channel quarters (4) -> 24 channels/core.
No cross-core communication.

Per-core layout: 4 channels x 32 leading-spatial -> 128 SBUF partitions,
1024-wide free dim (the remaining two spatial axes). Forward scan pairs
load as one 1 MiB DMA ([128, 2048], scan index in the free dim); the
flipped pairs load partition-block-reversed (negative i stride) as two
512 KiB DMAs, with the remaining free-dim reversal folded into the
combining add's access pattern. perm_j / perm_k are DVE 32x32 block
transposes plus free-dim (a,b)->(b,a) permuted access patterns. Loads
are spread across both HWDGE rings (SP + ACT).
"""

import numpy as np

_B, _S, _C, _D = 2, 12, 96, 32
_NCORE = 8
_CL = _C // 4          # 24 channels per core
_G = _CL // 4          # 6 macro tiles of 4 channels (128 partitions)
_FREE = _D * _D        # 1024

_nc = None


def _build_program():
    from concourse import bacc, tile, mybir

    f32 = mybir.dt.float32
    nc = bacc.Bacc(
        "TRN2", target_bir_lowering=False, debug=False, num_devices=_NCORE
    )
    ys = nc.dram_tensor("ys", [_S, _CL, _D, _D, _D], f32, kind="ExternalInput")
    out = nc.dram_tensor("out", [_CL, _D, _D, _D], f32, kind="ExternalOutput")
    ysa = ys.ap()
    outa = out.ap()

    with tile.TileContext(nc) as tc:
        with (
            tc.tile_pool(name="io", bufs=2) as iop,
            tc.tile_pool(name="tmp", bufs=2) as tmp,
        ):
            for g in range(_G):
                cs = slice(4 * g, 4 * (g + 1))

                def fwd_pair(s, tag):
                    # scans s, s+1 in one 1 MiB DMA; free dim = (scan, j*k)
                    t = iop.tile([128, 2 * _FREE], f32, tag=tag, name=tag)
                    src = ysa[s:s + 2, cs].rearrange(
                        "s c i j k -> (c i) s (j k)"
                    )
                    dst = t[:].rearrange("p (s f) -> p s f", s=2)
                    nc.sync.dma_start(out=dst, in_=src)
                    return t

                def rev_scan(s, tag):
                    # leading spatial axis reversed; (c, i) cannot merge
                    t = iop.tile([128, _FREE], f32, tag=tag, name=tag)
                    src = ysa[s, cs][:, ::-1].rearrange(
                        "c i j k -> c i (j k)"
                    )
                    nc.scalar.dma_start(out=t[:], in_=src)
                    return t

                af = fwd_pair(0, "af")
                ar0 = rev_scan(2, "ar0")
                ar1 = rev_scan(3, "ar1")
                bf = fwd_pair(4, "bf")
                br0 = rev_scan(6, "br0")
                br1 = rev_scan(7, "br1")
                cf = fwd_pair(8, "cf")
                cr0 = rev_scan(10, "cr0")
                cr1 = rev_scan(11, "cr1")

                def p3(ap):
                    return ap.rearrange("p (a b) -> p a b", a=_D)

                def pswap(ap):
                    return p3(ap).transpose([0, 2, 1])

                def halves(t):
                    return t[:, 0:_FREE], t[:, _FREE:2 * _FREE]

                def rrev(t):
                    return t[:][:, ::-1]

                # group 0 (volume already (i,j,k)): acc = fwd + flip(rev)
                acc = tmp.tile([128, _FREE], f32, tag="acc", name="acc")
                ry = tmp.tile([128, _FREE], f32, tag="ry", name="ry", bufs=6)
                h0, h1 = halves(af)
                nc.vector.tensor_add(acc[:], h0, h1)
                nc.vector.tensor_add(ry[:], rrev(ar0), rrev(ar1))
                nc.vector.tensor_add(acc[:], acc[:], ry[:])

                # group 1 ((j,k,i) order): combine, 32x32 block transpose,
                # then add with (k,j)->(j,k) free permute
                xb = tmp.tile([128, _FREE], f32, tag="xb", name="xb")
                ry = tmp.tile([128, _FREE], f32, tag="ry", name="ry", bufs=6)
                h0, h1 = halves(bf)
                nc.vector.tensor_add(xb[:], h0, h1)
                nc.vector.tensor_add(ry[:], rrev(br0), rrev(br1))
                nc.vector.tensor_add(xb[:], xb[:], ry[:])
                tb = tmp.tile([128, _FREE], f32, tag="tb", name="tb")
                nc.vector.transpose(tb[:], xb[:])
                nc.vector.tensor_add(p3(acc[:]), p3(acc[:]), pswap(tb[:]))

                # group 2 ((k,i,j) order): combine, (i,j)->(j,i) free
                # permute (ScalarE), then 32x32 block transpose
                xc = tmp.tile([128, _FREE], f32, tag="xc", name="xc")
                ry = tmp.tile([128, _FREE], f32, tag="ry", name="ry", bufs=6)
                h0, h1 = halves(cf)
                nc.vector.tensor_add(xc[:], h0, h1)
                nc.vector.tensor_add(ry[:], rrev(cr0), rrev(cr1))
                nc.vector.tensor_add(xc[:], xc[:], ry[:])
                cp = tmp.tile([128, _FREE], f32, tag="cp", name="cp")
                nc.scalar.copy(p3(cp[:]), pswap(xc[:]))
                tcb = tmp.tile([128, _FREE], f32, tag="tcb", name="tcb")
                nc.vector.transpose(tcb[:], cp[:])
                nc.vector.tensor_add(acc[:], acc[:], tcb[:])

                o = tmp.tile([128, _FREE], f32, tag="o", name="o")
                nc.scalar.mul(o[:], acc[:], 1.0 / 12.0)
                nc.sync.dma_start(
                    out=outa[cs].rearrange("c i j k -> (c i) (j k)"), in_=o[:]
                )

    nc.compile()
    return nc


def kernel(ys):
    global _nc
    ys = np.ascontiguousarray(ys, dtype=np.float32)
    assert ys.shape == (_B, _S, _C, _D, _D, _D), ys.shape

    if _nc is None:
        _nc = _build_program()

    from concourse.bass_utils import run_bass_kernel_spmd

    in_maps = []
    for r in range(_NCORE):
        b, q = divmod(r, 4)
        shard = np.ascontiguousarray(ys[b, :, q * _CL:(q + 1) * _CL])
        in_maps.append({"ys": shard})

    res = run_bass_kernel_spmd(_nc, in_maps, list(range(_NCORE)))

    out = np.empty((_B, _C, _D, _D, _D), np.float32)
    for r in range(_NCORE):
        b, q = divmod(r, 4)
        out[b, q * _CL:(q + 1) * _CL] = res.results[r]["out"]

    if res.exec_time_ns is not None:
        print(f"HW exec time: {res.exec_time_ns} ns")
    return out


# revision 6
# speedup vs baseline: 1.5402x; 1.4898x over previous
"""Trainium2 Bass kernel for CrossMerge3D.

Input ys: [B=2, S=12, C=96, 32, 32, 32] f32. For each (b, c):
  out = (m0 + perm_j(m1) + perm_k(m2)) / 12
where, with the 12 scans split into 3 groups of 4, each group combines as
  m_g = s0 + s1 + flip(s2 + s3)   (flip over the flattened 32^3 volume)
and group 1's volume is stored as (j,k,i), group 2's as (k,i,j); perm_j /
perm_k bring them back to (i,j,k).

Sharding: 8 cores = batch (2) x channel quarters (4) -> 24 channels/core.
No cross-core communication.

Per-core layout: 4 channels x 32 leading-spatial -> 128 SBUF partitions,
1024-wide free dim (remaining two spatial axes). All loads are plain
mergeable scan-pair DMAs (1 MiB, fast HWDGE descriptor path — reversed /
multi-dim source APs cost ~6.4us per trigger on the issuing sequencer,
vs ~0.65us for these). The flip's partition-block reversal runs on the
otherwise-idle TensorEngine as a matmul against a block-exchange matrix
(fp32 PE matmuls are bit-exact here), with the free-dim reversal folded
into the moving operand's AP; the same PSUM accumulates all 12 scans.
perm_j / perm_k are DVE 32x32 block transposes plus free-dim permuted
APs on the PSUM-accumulating matmuls.
"""

import numpy as np

_B, _S, _C, _D = 2, 12, 96, 32
_NCORE = 8
_CL = _C // 4          # 24 channels per core
_G = _CL // 4          # 6 macro tiles of 4 channels (128 partitions)
_FREE = _D * _D        # 1024

_nc = None


def _build_program():
    from concourse import bacc, tile, mybir

    f32 = mybir.dt.float32
    nc = bacc.Bacc(
        "TRN2", target_bir_lowering=False, debug=False, num_devices=_NCORE
    )
    ys = nc.dram_tensor("ys", [_S, _CL, _D, _D, _D], f32, kind="ExternalInput")
    out = nc.dram_tensor("out", [_CL, _D, _D, _D], f32, kind="ExternalOutput")
    ysa = ys.ap()
    outa = out.ap()

    with tile.TileContext(nc) as tc:
        with (
            tc.tile_pool(name="const", bufs=1) as cst,
            tc.tile_pool(name="io", bufs=2) as iop,
            tc.tile_pool(name="tmp", bufs=2) as tmp,
            tc.tile_pool(name="ps", bufs=1, space="PSUM") as ps,
        ):
            # stationaries: identity and 32-block exchange (anti-diagonal
            # per 32-partition block), built once via affine_select
            ident = cst.tile([128, 128], f32, tag="ident", name="ident")
            nc.gpsimd.memset(ident[:], 1.0)
            nc.gpsimd.affine_select(
                out=ident[:], in_=ident[:],
                compare_op=mybir.AluOpType.is_equal, fill=0.0,
                base=0, pattern=[[-1, 128]], channel_multiplier=1,
            )
            jblk = cst.tile([128, 128], f32, tag="jblk", name="jblk")
            nc.gpsimd.memset(jblk[:], 1.0)
            for b in range(4):
                # slice rows r (absolute p = 32b + r): keep f == 32b + 31 - r
                nc.gpsimd.affine_select(
                    out=jblk[32 * b:32 * b + 32, :],
                    in_=jblk[32 * b:32 * b + 32, :],
                    compare_op=mybir.AluOpType.is_equal, fill=0.0,
                    base=-(32 * b + 31), pattern=[[1, 128]],
                    channel_multiplier=1,
                )

            for g in range(_G):
                cs = slice(4 * g, 4 * (g + 1))

                def load_pair(s, tag, eng):
                    # scans s, s+1 in one 1 MiB DMA; free dim = (scan, j*k)
                    t = iop.tile([128, 2 * _FREE], f32, tag=tag, name=tag)
                    src = ysa[s:s + 2, cs].rearrange(
                        "s c i j k -> (c i) s (j k)"
                    )
                    dst = t[:].rearrange("p (s f) -> p s f", s=2)
                    eng.dma_start(out=dst, in_=src)
                    return t

                pa = load_pair(0, "pa", nc.sync)
                pr = load_pair(2, "pr", nc.scalar)
                qa = load_pair(4, "qa", nc.sync)
                qr = load_pair(6, "qr", nc.scalar)
                ra = load_pair(8, "ra", nc.sync)
                rr = load_pair(10, "rr", nc.scalar)

                def group_mms(pt, fwd, rev, start, stop):
                    # pt += fwd[s0] + fwd[s1] + flip(rev[s0]) + flip(rev[s1]);
                    # flip = partition-block reversal (jblk) + free-dim
                    # reversal (negative-stride moving AP)
                    f0, f1 = fwd[:, 0:_FREE], fwd[:, _FREE:2 * _FREE]
                    r0 = rev[:, 0:_FREE][:, ::-1]
                    r1 = rev[:, _FREE:2 * _FREE][:, ::-1]
                    for n0 in (0, 512):
                        sl = slice(n0, n0 + 512)
                        nc.tensor.matmul(pt[:, sl], ident[:], f0[:, sl],
                                         start=start, stop=False)
                        nc.tensor.matmul(pt[:, sl], ident[:], f1[:, sl],
                                         start=False, stop=False)
                        nc.tensor.matmul(pt[:, sl], jblk[:], r0[:, sl],
                                         start=False, stop=False)
                        nc.tensor.matmul(pt[:, sl], jblk[:], r1[:, sl],
                                         start=False, stop=stop)

                # group 0 accumulates everything; B/C merge via transposes
                psA = ps.tile([128, _FREE], f32, tag="psA", name="psA",
                              bufs=2)
                group_mms(psA, pa, pr, start=True, stop=False)

                # group 1 ((j,k,i) order)
                psB = ps.tile([128, _FREE], f32, tag="psBC", name="psB",
                              bufs=2)
                group_mms(psB, qa, qr, start=True, stop=True)
                xb = tmp.tile([128, _FREE], f32, tag="xb", name="xb")
                nc.scalar.copy(xb[:], psB[:])
                tb = tmp.tile([128, _FREE], f32, tag="tb", name="tb")
                nc.vector.transpose(tb[:], xb[:])
                # accumulate perm_j: psA[p, j*32+k] += tb[p, k*32+j]
                tbp = tb[:].rearrange("p (a b) -> p a b", a=_D).transpose(
                    [0, 2, 1]
                )
                nc.tensor.matmul(psA[:, 0:512], ident[:], tbp[:, 0:16],
                                 start=False, stop=False)
                nc.tensor.matmul(psA[:, 512:1024], ident[:], tbp[:, 16:32],
                                 start=False, stop=False)

                # group 2 ((k,i,j) order)
                psC = ps.tile([128, _FREE], f32, tag="psBC", name="psC",
                              bufs=2)
                group_mms(psC, ra, rr, start=True, stop=True)
                # copy out with (i,j)->(j,i) free permute, then block-transpose
                cp = tmp.tile([128, _FREE], f32, tag="cp", name="cp")
                pcs = psC[:].rearrange("p (a b) -> p a b", a=_D).transpose(
                    [0, 2, 1]
                )
                nc.scalar.copy(cp[:].rearrange("p (a b) -> p a b", a=_D), pcs)
                tcb = tmp.tile([128, _FREE], f32, tag="tcb", name="tcb")
                nc.vector.transpose(tcb[:], cp[:])
                nc.tensor.matmul(psA[:, 0:512], ident[:], tcb[:, 0:512],
                                 start=False, stop=True)
                nc.tensor.matmul(psA[:, 512:1024], ident[:], tcb[:, 512:1024],
                                 start=False, stop=True)

                o = tmp.tile([128, _FREE], f32, tag="o", name="o")
                nc.scalar.mul(o[:], psA[:], 1.0 / 12.0)
                nc.sync.dma_start(
                    out=outa[cs].rearrange("c i j k -> (c i) (j k)"), in_=o[:]
                )

    nc.compile()
    return nc


def kernel(ys):
    global _nc
    ys = np.ascontiguousarray(ys, dtype=np.float32)
    assert ys.shape == (_B, _S, _C, _D, _D, _D), ys.shape

    if _nc is None:
        _nc = _build_program()

    from concourse.bass_utils import run_bass_kernel_spmd

    in_maps = []
    for r in range(_NCORE):
        b, q = divmod(r, 4)
        shard = np.ascontiguousarray(ys[b, :, q * _CL:(q + 1) * _CL])
        in_maps.append({"ys": shard})

    res = run_bass_kernel_spmd(_nc, in_maps, list(range(_NCORE)))

    out = np.empty((_B, _C, _D, _D, _D), np.float32)
    for r in range(_NCORE):
        b, q = divmod(r, 4)
        out[b, q * _CL:(q + 1) * _CL] = res.results[r]["out"]

    if res.exec_time_ns is not None:
        print(f"HW exec time: {res.exec_time_ns} ns")
    return out


# revision 8
# speedup vs baseline: 1.9944x; 1.2949x over previous
"""Trainium2 Bass kernel for CrossMerge3D.

Input ys: [B=2, S=12, C=96, 32, 32, 32] f32. For each (b, c):
  out = (m0 + perm_j(m1) + perm_k(m2)) / 12
where, with the 12 scans split into 3 groups of 4, each group combines as
  m_g = s0 + s1 + flip(s2 + s3)   (flip over the flattened 32^3 volume)
and group 1's volume is stored as (j,k,i), group 2's as (k,i,j); perm_j /
perm_k bring them back to (i,j,k).

Sharding: 8 cores = batch (2) x channel quarters (4) -> 24 channels/core.
No cross-core communication.

Per-core layout: 4 channels x 32 leading-spatial -> 128 SBUF partitions,
1024-wide free dim. All loads are plain mergeable scan-pair DMAs (1 MiB,
fast HWDGE descriptor path; reversed/multi-dim source APs cost ~6.4us
per trigger on the issuing sequencer vs ~0.65us for these). The flip
splits into a free-dim reversal (folded into the pair-sum's operand APs)
and a partition-block reversal (one bit-exact fp32 matmul against a
block-exchange matrix on the otherwise idle TensorEngine). Elementwise
work is spread across DVE and GPSIMD so no engine exceeds the DMA
roofline. perm_j / perm_k are DVE 32x32 block transposes plus free-dim
permuted APs.
"""

import numpy as np

_B, _S, _C, _D = 2, 12, 96, 32
_NCORE = 8
_CL = _C // 4          # 24 channels per core
_G = _CL // 4          # 6 macro tiles of 4 channels (128 partitions)
_FREE = _D * _D        # 1024

_nc = None


def _build_program():
    from concourse import bacc, tile, mybir

    f32 = mybir.dt.float32
    nc = bacc.Bacc(
        "TRN2", target_bir_lowering=False, debug=False, num_devices=_NCORE
    )
    ys = nc.dram_tensor("ys", [_S, _CL, _D, _D, _D], f32, kind="ExternalInput")
    out = nc.dram_tensor("out", [_CL, _D, _D, _D], f32, kind="ExternalOutput")
    ysa = ys.ap()
    outa = out.ap()

    with tile.TileContext(nc) as tc:
        with (
            tc.tile_pool(name="const", bufs=1) as cst,
            tc.tile_pool(name="io", bufs=2) as iop,
            tc.tile_pool(name="tmp", bufs=2) as tmp,
            tc.tile_pool(name="ps", bufs=1, space="PSUM") as ps,
        ):
            # 32-block exchange stationary (anti-diagonal per block)
            jblk = cst.tile([128, 128], f32, tag="jblk", name="jblk")
            nc.gpsimd.memset(jblk[:], 1.0)
            for b in range(4):
                nc.gpsimd.affine_select(
                    out=jblk[32 * b:32 * b + 32, :],
                    in_=jblk[32 * b:32 * b + 32, :],
                    compare_op=mybir.AluOpType.is_equal, fill=0.0,
                    base=-(32 * b + 31), pattern=[[1, 128]],
                    channel_multiplier=1,
                )

            for g in range(_G):
                cs = slice(4 * g, 4 * (g + 1))

                def load_pair(s, tag, eng):
                    t = iop.tile([128, 2 * _FREE], f32, tag=tag, name=tag)
                    src = ysa[s:s + 2, cs].rearrange(
                        "s c i j k -> (c i) s (j k)"
                    )
                    dst = t[:].rearrange("p (s f) -> p s f", s=2)
                    eng.dma_start(out=dst, in_=src)
                    return t

                pa = load_pair(0, "pa", nc.sync)
                pr = load_pair(2, "pr", nc.scalar)
                qa = load_pair(4, "qa", nc.sync)
                qr = load_pair(6, "qr", nc.scalar)
                ra = load_pair(8, "ra", nc.sync)
                rr = load_pair(10, "rr", nc.scalar)

                def fwd_sum(t, eng):
                    # in-place into the first half (elementwise aligned)
                    h0, h1 = t[:, 0:_FREE], t[:, _FREE:2 * _FREE]
                    eng.tensor_add(h0, h0, h1)
                    return h0

                def rev_sum(t, tag, eng):
                    # free-dim-reversed pair sum; partition reversal is done
                    # later by the jblk matmul
                    rs = tmp.tile([128, _FREE], f32, tag=tag, name=tag)
                    eng.tensor_add(rs[:], t[:, 0:_FREE][:, ::-1],
                                   t[:, _FREE:2 * _FREE][:, ::-1])
                    return rs

                fA = fwd_sum(pa, nc.vector)
                rA = rev_sum(pr, "rA", nc.vector)
                fB = fwd_sum(qa, nc.vector)
                rB = rev_sum(qr, "rB", nc.vector)
                fC = fwd_sum(ra, nc.vector)
                rC = rev_sum(rr, "rC", nc.vector)

                def flip(rs, name):
                    # partition-block reversal on the TensorEngine
                    pf = ps.tile([128, _FREE], f32, tag="psF", name=name,
                                 bufs=4)
                    for n0 in (0, 512):
                        nc.tensor.matmul(pf[:, n0:n0 + 512], jblk[:],
                                         rs[:][:, n0:n0 + 512],
                                         start=True, stop=True)
                    return pf

                pfA = flip(rA, "pfA")
                pfB = flip(rB, "pfB")
                pfC = flip(rC, "pfC")

                # combines: grp = fwd + flipped_rev (PSUM operand)
                nc.vector.tensor_add(rA[:], fA, pfA[:])
                nc.vector.tensor_add(rB[:], fB, pfB[:])
                nc.vector.tensor_add(rC[:], fC, pfC[:])

                # group 1 ((j,k,i)): 32x32 block transpose, then add with
                # (k,j)->(j,k) free permute
                tb = tmp.tile([128, _FREE], f32, tag="tb", name="tb")
                nc.vector.transpose(tb[:], rB[:])
                acc3 = rA[:].rearrange("p (a b) -> p a b", a=_D)
                tbp = tb[:].rearrange("p (a b) -> p a b", a=_D).transpose(
                    [0, 2, 1]
                )
                nc.vector.tensor_add(acc3, acc3, tbp)

                # group 2 ((k,i,j)): (i,j)->(j,i) free permute (ScalarE),
                # then 32x32 block transpose
                cp = tmp.tile([128, _FREE], f32, tag="cp", name="cp")
                rcp = rC[:].rearrange("p (a b) -> p a b", a=_D).transpose(
                    [0, 2, 1]
                )
                nc.scalar.copy(cp[:].rearrange("p (a b) -> p a b", a=_D), rcp)
                tcb = tmp.tile([128, _FREE], f32, tag="tcb", name="tcb")
                nc.vector.transpose(tcb[:], cp[:])
                nc.vector.tensor_add(rA[:], rA[:], tcb[:])

                o = tmp.tile([128, _FREE], f32, tag="o", name="o")
                nc.scalar.mul(o[:], rA[:], 1.0 / 12.0)
                nc.sync.dma_start(
                    out=outa[cs].rearrange("c i j k -> (c i) (j k)"), in_=o[:]
                )

    nc.compile()
    return nc


def kernel(ys):
    global _nc
    ys = np.ascontiguousarray(ys, dtype=np.float32)
    assert ys.shape == (_B, _S, _C, _D, _D, _D), ys.shape

    if _nc is None:
        _nc = _build_program()

    from concourse.bass_utils import run_bass_kernel_spmd

    in_maps = []
    for r in range(_NCORE):
        b, q = divmod(r, 4)
        shard = np.ascontiguousarray(ys[b, :, q * _CL:(q + 1) * _CL])
        in_maps.append({"ys": shard})

    res = run_bass_kernel_spmd(_nc, in_maps, list(range(_NCORE)))

    out = np.empty((_B, _C, _D, _D, _D), np.float32)
    for r in range(_NCORE):
        b, q = divmod(r, 4)
        out[b, q * _CL:(q + 1) * _CL] = res.results[r]["out"]

    if res.exec_time_ns is not None:
        print(f"HW exec time: {res.exec_time_ns} ns")
    return out
